# revision 7
# baseline (speedup 1.0000x reference)
"""Trainium2 Bass/Tile kernel: single-head attention (B=8, S=2048, E=1024, DQ=DV=128).

Data-parallel over the batch: one batch element per NeuronCore (8 cores), no
collectives. Host pre-transposes activations to [E, S] bf16 so the contraction
dim lands on SBUF partitions; everything else runs on-chip:

  qT/kT/vT = W.T @ xT          (PE, bf16 in / fp32 PSUM accum, bias on ACT copy)
  v_aug    = transpose(vT) ++ ones column   (PE transpose; ones give row sums)
  scoresT  = kT_chunk.T @ qT   ([keys, queries] layout; causal upper blocks skipped)
  attnT    = exp(scoresT/sqrt(DQ) + pad_bias)  (ACT; pad mask is a per-partition bias;
                                               no max-subtraction needed: |scores| < ~3)
  out[q,:] = (attnT.T @ v_aug)[:, :DV] * recip(row_sum)   (PE + DVE recip + ACT scale)
"""

import numpy as np
import ml_dtypes
from contextlib import ExitStack

B, S, E, DQ, DV = 8, 2048, 1024, 128, 128
EC = E // 128    # contraction chunks
SC = S // 128    # sequence chunks
QB = 512         # matmul moving-dim block
RSQRT_DQ = 1.0 / float(np.sqrt(DQ))
NEG = np.float32(-1e9)
_BF16 = ml_dtypes.bfloat16

_prog = None


def _build_program():
    import concourse.bacc as bacc
    import concourse.mybir as mybir
    import concourse.tile as tile

    f32 = mybir.dt.float32
    bf16 = mybir.dt.bfloat16
    AF = mybir.ActivationFunctionType

    nc = bacc.Bacc("TRN2", target_bir_lowering=False, debug=False)

    d_x = {n: nc.dram_tensor(n, [E, S], bf16, kind="ExternalInput").ap()
           for n in ("qT", "kT", "vT")}
    d_w = {n: nc.dram_tensor(n, [128, EC, 128], bf16, kind="ExternalInput").ap()
           for n in ("wq", "wk", "wv")}
    d_b = {n: nc.dram_tensor(n, [128, 1], f32, kind="ExternalInput").ap()
           for n in ("bq", "bk", "bv")}
    d_padb = nc.dram_tensor("padb", [128, SC], f32, kind="ExternalInput").ap()
    d_tri = nc.dram_tensor("tri", [128, 128], bf16, kind="ExternalInput").ap()
    d_eye = nc.dram_tensor("eye", [128, 128], bf16, kind="ExternalInput").ap()
    d_out = nc.dram_tensor("out", [S, DV], f32, kind="ExternalOutput").ap()

    with tile.TileContext(nc) as tc, ExitStack() as ctx:
        consts = ctx.enter_context(tc.tile_pool(name="consts", bufs=1))
        xin_p = ctx.enter_context(tc.tile_pool(name="xin", bufs=EC + 1))
        proj_p = ctx.enter_context(tc.tile_pool(name="proj", bufs=1))
        attn_p = ctx.enter_context(tc.tile_pool(name="attn", bufs=1))
        out_p = ctx.enter_context(tc.tile_pool(name="outp", bufs=3))
        ps_main = ctx.enter_context(tc.tile_pool(name="ps_main", bufs=4, space="PSUM"))
        ps_av = ctx.enter_context(tc.tile_pool(name="ps_av", bufs=2, space="PSUM"))

        w_sb = {}
        for n in ("wq", "wk", "wv"):
            t = consts.tile([128, EC, 128], bf16, tag=n)
            nc.gpsimd.dma_start(t[:, :, :], d_w[n])
            w_sb[n] = t
        b_sb = {}
        for n in ("bq", "bk", "bv"):
            t = consts.tile([128, 1], f32, tag=n)
            nc.gpsimd.dma_start(t[:, :], d_b[n])
            b_sb[n] = t
        padb = consts.tile([128, SC], f32, tag="padb")
        nc.gpsimd.dma_start(padb[:, :], d_padb)
        tri = consts.tile([128, 128], bf16, tag="tri")
        nc.gpsimd.dma_start(tri[:, :], d_tri)
        eye = consts.tile([128, 128], bf16, tag="eye")
        nc.gpsimd.dma_start(eye[:, :], d_eye)

        # One-time ACT LUT loads, hidden under the first input DMAs.
        warm = consts.tile([128, 1], f32, tag="warm")
        nc.vector.memset(warm[:, :], 0.0)
        wo = consts.tile([128, 1], f32, tag="warmo")
        nc.scalar.activation(wo[:, :], warm[:, :], AF.Exp)
        nc.scalar.activation(wo[:, :], warm[:, :], AF.Identity)
        nc.scalar.mul(wo[:, :], warm[:, :], 1.0)

        # ---- Projections: xT [E, S] -> qT/kT/vT [128(dq/dv), S] bf16 ----
        qT = proj_p.tile([128, S], bf16, tag="qT")
        kT = proj_p.tile([128, S], bf16, tag="kT")
        vT = proj_p.tile([128, S], bf16, tag="vT")
        for dx, w, bias, dst in ((d_x["qT"], w_sb["wq"], b_sb["bq"], qT),
                                 (d_x["kT"], w_sb["wk"], b_sb["bk"], kT),
                                 (d_x["vT"], w_sb["wv"], b_sb["bv"], vT)):
            xin = []
            for c in range(EC):
                t = xin_p.tile([128, S], bf16, tag="xin")
                nc.gpsimd.dma_start(t[:, :], dx[c * 128:(c + 1) * 128, :])
                xin.append(t)
            for n0 in range(0, S, QB):
                ps = ps_main.tile([128, QB], f32, tag="ps")
                for c in range(EC):
                    nc.tensor.matmul(ps[:, :], w[:, c, :], xin[c][:, n0:n0 + QB],
                                     start=(c == 0), stop=(c == EC - 1))
                nc.scalar.activation(dst[:, n0:n0 + QB], ps[:, :], AF.Identity,
                                     bias=bias[:, :])

        # ---- v_aug[j] = v natural [keys, DV] ++ ones column ----
        vaug = []
        for j in range(SC):
            ps = ps_main.tile([128, 128], bf16, tag="ps_t", bufs=2)
            nc.tensor.transpose(ps[:, :], vT[:, j * 128:(j + 1) * 128], eye[:, :])
            va = attn_p.tile([128, DV + 1], bf16, tag=f"vaug{j}")
            nc.scalar.activation(va[:, 0:DV], ps[:, :], AF.Copy)
            nc.vector.memset(va[:, DV:DV + 1], 1.0)
            vaug.append(va)

        # ---- scoresT -> exp -> attnT bf16, per key chunk j (causal: q >= j*128) ----
        attnT = []
        for j in range(SC):
            at = attn_p.tile([128, S - j * 128], bf16, tag=f"attnT{j}")
            attnT.append(at)
            q0 = j * 128
            while q0 < S:
                n = min(QB, S - q0)
                ps = ps_main.tile([128, n], f32, tag="ps")
                nc.tensor.matmul(ps[:, :], kT[:, j * 128:(j + 1) * 128],
                                 qT[:, q0:q0 + n], start=True, stop=True)
                nc.scalar.activation(at[:, q0 - j * 128:q0 - j * 128 + n], ps[:, :],
                                     AF.Exp, bias=padb[:, j:j + 1], scale=RSQRT_DQ)
                q0 += n
            # in-block causal mask on the diagonal block (keep k <= q)
            nc.vector.tensor_mul(at[:, 0:128], at[:, 0:128], tri[:, :])

        # ---- AV per q tile + fused normalization ----
        for i in range(SC):
            ps = ps_av.tile([128, DV + 1], f32, tag="pso")
            for j in range(i + 1):
                nc.tensor.matmul(ps[:, :],
                                 attnT[j][:, (i - j) * 128:(i - j) * 128 + 128],
                                 vaug[j][:, :], start=(j == 0), stop=(j == i))
            rec = out_p.tile([128, 1], f32, tag="rec")
            nc.vector.reciprocal(rec[:, :], ps[:, DV:DV + 1])
            ot = out_p.tile([128, DV], f32, tag="ot")
            nc.scalar.mul(ot[:, :], ps[:, 0:DV], rec[:, :])
            nc.gpsimd.dma_start(d_out[i * 128:(i + 1) * 128, :], ot[:, :])

    nc.compile()
    return nc


def _prep_inputs(pad_mask, query, key, value, Wq, bq, Wk, bk, Wv, bv):
    def wprep(w):
        return np.ascontiguousarray(
            np.asarray(w, np.float32).astype(_BF16).reshape(EC, 128, 128)
            .transpose(1, 0, 2))

    def bprep(v):
        return np.ascontiguousarray(np.asarray(v, np.float32).reshape(128, 1))

    shared = {
        "wq": wprep(Wq), "wk": wprep(Wk), "wv": wprep(Wv),
        "bq": bprep(bq), "bk": bprep(bk), "bv": bprep(bv),
        "tri": np.triu(np.ones((128, 128), np.float32)).astype(_BF16),
        "eye": np.eye(128, dtype=np.float32).astype(_BF16),
    }
    pad_mask = np.asarray(pad_mask)
    query = np.asarray(query, np.float32)
    key = np.asarray(key, np.float32)
    value = np.asarray(value, np.float32)
    in_maps = []
    for b in range(B):
        padb = np.ascontiguousarray(
            np.where(pad_mask[b], NEG, np.float32(0.0)).reshape(SC, 128).T)
        in_maps.append({
            **shared,
            "qT": query[b].T.astype(_BF16, order="C"),
            "kT": key[b].T.astype(_BF16, order="C"),
            "vT": value[b].T.astype(_BF16, order="C"),
            "padb": padb.astype(np.float32),
        })
    return in_maps


def _run(in_maps, trace=False, **kwargs):
    global _prog
    from concourse.bass_utils import run_bass_kernel_spmd
    if _prog is None:
        _prog = _build_program()
    return run_bass_kernel_spmd(_prog, in_maps, list(range(B)), trace=trace,
                                **kwargs)

def kernel(pad_mask, query, key, value, Wq, bq, Wk, bk, Wv, bv):
    in_maps = _prep_inputs(pad_mask, query, key, value, Wq, bq, Wk, bk, Wv, bv)
    res = _run(in_maps)
    out = np.stack([np.asarray(res.results[i]["out"]) for i in range(B)])
    return np.ascontiguousarray(out.astype(np.float32))


# revision 8
# speedup vs baseline: 1.0094x; 1.0094x over previous
"""Trainium2 Bass/Tile kernel: single-head attention (B=8, S=2048, E=1024, DQ=DV=128).

Data-parallel over the batch: one batch element per NeuronCore (8 cores), no
collectives. Host pre-transposes activations to [E, S] bf16 so the contraction
dim lands on SBUF partitions; everything else runs on-chip:

  qT/kT/vT = W.T @ xT          (PE, bf16 in / fp32 PSUM accum, bias added on DVE copy)
  v_aug    = transpose(vT) ++ ones column   (PE transpose; ones column makes the
                                             AV matmul emit softmax row sums for free)
  scoresT  = kT_chunk.T @ qT   ([keys, queries] layout; causal upper blocks skipped)
  attnT    = exp(scoresT/sqrt(DQ) + pad_bias)  (ACT; pad mask is a per-partition bias;
                                               no max-subtraction needed: |scores| < ~3)
  out[q,:] = (attnT.T @ v_aug)[:, :DV] * recip(row_sum)   (PE + DVE recip/scale)

Engine budget per core: PE ~45us, DMA ~36us (13.4MB), ACT ~21us (exp only),
DVE ~19us (copies, masks, normalize).
"""

import numpy as np
import ml_dtypes
from contextlib import ExitStack

B, S, E, DQ, DV = 8, 2048, 1024, 128, 128
EC = E // 128    # contraction chunks
SC = S // 128    # sequence chunks
QB = 512         # matmul moving-dim block
XB = 1024        # exp batching width (2 PSUM banks)
RSQRT_DQ = 1.0 / float(np.sqrt(DQ))
NEG = np.float32(-1e9)
_BF16 = ml_dtypes.bfloat16

_prog = None


def _build_program():
    import concourse.bacc as bacc
    import concourse.mybir as mybir
    import concourse.tile as tile

    f32 = mybir.dt.float32
    bf16 = mybir.dt.bfloat16
    AF = mybir.ActivationFunctionType
    ALU = mybir.AluOpType

    nc = bacc.Bacc("TRN2", target_bir_lowering=False, debug=False)

    d_x = {n: nc.dram_tensor(n, [E, S], bf16, kind="ExternalInput").ap()
           for n in ("qT", "kT", "vT")}
    d_w = {n: nc.dram_tensor(n, [128, EC, 128], bf16, kind="ExternalInput").ap()
           for n in ("wq", "wk", "wv")}
    d_b = {n: nc.dram_tensor(n, [128, 1], f32, kind="ExternalInput").ap()
           for n in ("bq", "bk", "bv")}
    d_padb = nc.dram_tensor("padb", [128, SC], f32, kind="ExternalInput").ap()
    d_tri = nc.dram_tensor("tri", [128, 128], bf16, kind="ExternalInput").ap()
    d_eye = nc.dram_tensor("eye", [128, 128], bf16, kind="ExternalInput").ap()
    d_out = nc.dram_tensor("out", [S, DV], f32, kind="ExternalOutput").ap()

    with tile.TileContext(nc) as tc, ExitStack() as ctx:
        consts = ctx.enter_context(tc.tile_pool(name="consts", bufs=1))
        xin_p = ctx.enter_context(tc.tile_pool(name="xin", bufs=EC + 1))
        proj_p = ctx.enter_context(tc.tile_pool(name="proj", bufs=1))
        attn_p = ctx.enter_context(tc.tile_pool(name="attn", bufs=1))
        out_p = ctx.enter_context(tc.tile_pool(name="outp", bufs=3))
        ps_main = ctx.enter_context(tc.tile_pool(name="ps_main", bufs=2, space="PSUM"))
        ps_av = ctx.enter_context(tc.tile_pool(name="ps_av", bufs=2, space="PSUM"))

        # Big input streams first (HWDGE via sync engine) so the first
        # projection matmul starts ~2us in, not behind const loads.
        xin = {}
        for name in ("qT", "kT", "vT"):
            xin[name] = []
            for c in range(EC):
                t = xin_p.tile([128, S], bf16, tag="xin")
                nc.sync.dma_start(t[:, :], d_x[name][c * 128:(c + 1) * 128, :])
                xin[name].append(t)

        # Constants ride the SWDGE (gpsimd) path, off the HWDGE rings.
        w_sb = {}
        for n in ("wq", "wk", "wv"):
            t = consts.tile([128, EC, 128], bf16, tag=n)
            nc.gpsimd.dma_start(t[:, :, :], d_w[n])
            w_sb[n] = t
        b_sb = {}
        for n in ("bq", "bk", "bv"):
            t = consts.tile([128, 1], f32, tag=n)
            nc.gpsimd.dma_start(t[:, :], d_b[n])
            b_sb[n] = t
        padb = consts.tile([128, SC], f32, tag="padb")
        nc.gpsimd.dma_start(padb[:, :], d_padb)
        tri = consts.tile([128, 128], bf16, tag="tri")
        nc.gpsimd.dma_start(tri[:, :], d_tri)
        eye = consts.tile([128, 128], bf16, tag="eye")
        nc.gpsimd.dma_start(eye[:, :], d_eye)

        # One-time exp LUT load, hidden under the first input DMAs.
        warm = consts.tile([128, 1], f32, tag="warm")
        nc.vector.memset(warm[:, :], 0.0)
        wo = consts.tile([128, 1], f32, tag="warmo")
        nc.scalar.activation(wo[:, :], warm[:, :], AF.Exp)

        # ---- Projections: xT [E, S] -> qT/kT/vT [128(dq/dv), S] bf16 ----
        qT = proj_p.tile([128, S], bf16, tag="qT")
        kT = proj_p.tile([128, S], bf16, tag="kT")
        vT = proj_p.tile([128, S], bf16, tag="vT")
        for name, bias, dst in (("qT", b_sb["bq"], qT),
                                ("kT", b_sb["bk"], kT),
                                ("vT", b_sb["bv"], vT)):
            w = w_sb["w" + name[0]]
            for n0 in range(0, S, QB):
                ps = ps_main.tile([128, QB], f32, tag="ps")
                for c in range(EC):
                    nc.tensor.matmul(ps[:, :], w[:, c, :],
                                     xin[name][c][:, n0:n0 + QB],
                                     start=(c == 0), stop=(c == EC - 1))
                # copy + per-partition bias add + bf16 cast on DVE
                nc.vector.tensor_scalar(dst[:, n0:n0 + QB], ps[:, :],
                                        bias[:, :], None, ALU.add)

        # ---- v_aug[j] = v natural [keys, DV] ++ ones column ----
        vaug = []
        for j in range(SC):
            ps = ps_main.tile([128, 128], bf16, tag="ps_t")
            nc.tensor.transpose(ps[:, :], vT[:, j * 128:(j + 1) * 128], eye[:, :])
            va = attn_p.tile([128, DV + 1], bf16, tag=f"vaug{j}")
            nc.vector.tensor_copy(va[:, 0:DV], ps[:, :])
            nc.vector.memset(va[:, DV:DV + 1], 1.0)
            vaug.append(va)

        # ---- scoresT -> exp -> attnT bf16, per key chunk j (causal: q >= j*128) ----
        attnT = []
        for j in range(SC):
            at = attn_p.tile([128, S - j * 128], bf16, tag=f"attnT{j}")
            attnT.append(at)
            p0 = j * 128
            while p0 < S:
                n = min(XB, S - p0)
                ps = ps_main.tile([128, n], f32, tag="ps")
                for q0 in range(p0, p0 + n, QB):
                    m = min(QB, p0 + n - q0)
                    nc.tensor.matmul(ps[:, q0 - p0:q0 - p0 + m],
                                     kT[:, j * 128:(j + 1) * 128],
                                     qT[:, q0:q0 + m], start=True, stop=True)
                nc.scalar.activation(at[:, p0 - j * 128:p0 - j * 128 + n], ps[:, :],
                                     AF.Exp, bias=padb[:, j:j + 1], scale=RSQRT_DQ)
                p0 += n
            # in-block causal mask on the diagonal block (keep k <= q)
            nc.vector.tensor_mul(at[:, 0:128], at[:, 0:128], tri[:, :])

        # ---- AV per q tile + fused normalization ----
        for i in range(SC):
            ps = ps_av.tile([128, DV + 1], f32, tag="pso")
            for j in range(i + 1):
                nc.tensor.matmul(ps[:, :],
                                 attnT[j][:, (i - j) * 128:(i - j) * 128 + 128],
                                 vaug[j][:, :], start=(j == 0), stop=(j == i))
            rec = out_p.tile([128, 1], f32, tag="rec")
            nc.vector.reciprocal(rec[:, :], ps[:, DV:DV + 1])
            ot = out_p.tile([128, DV], f32, tag="ot")
            nc.vector.tensor_scalar(ot[:, :], ps[:, 0:DV], rec[:, :], None,
                                    ALU.mult)
            nc.scalar.dma_start(d_out[i * 128:(i + 1) * 128, :], ot[:, :])

    nc.compile()
    return nc


def _prep_inputs(pad_mask, query, key, value, Wq, bq, Wk, bk, Wv, bv):
    def wprep(w):
        return np.ascontiguousarray(
            np.asarray(w, np.float32).astype(_BF16).reshape(EC, 128, 128)
            .transpose(1, 0, 2))

    def bprep(v):
        return np.ascontiguousarray(np.asarray(v, np.float32).reshape(128, 1))

    shared = {
        "wq": wprep(Wq), "wk": wprep(Wk), "wv": wprep(Wv),
        "bq": bprep(bq), "bk": bprep(bk), "bv": bprep(bv),
        "tri": np.triu(np.ones((128, 128), np.float32)).astype(_BF16),
        "eye": np.eye(128, dtype=np.float32).astype(_BF16),
    }
    pad_mask = np.asarray(pad_mask)
    query = np.asarray(query, np.float32)
    key = np.asarray(key, np.float32)
    value = np.asarray(value, np.float32)
    in_maps = []
    for b in range(B):
        padb = np.ascontiguousarray(
            np.where(pad_mask[b], NEG, np.float32(0.0)).reshape(SC, 128).T)
        in_maps.append({
            **shared,
            "qT": query[b].T.astype(_BF16, order="C"),
            "kT": key[b].T.astype(_BF16, order="C"),
            "vT": value[b].T.astype(_BF16, order="C"),
            "padb": padb.astype(np.float32),
        })
    return in_maps


def _run(in_maps, trace=False, **kwargs):
    global _prog
    from concourse.bass_utils import run_bass_kernel_spmd
    if _prog is None:
        _prog = _build_program()
    return run_bass_kernel_spmd(_prog, in_maps, list(range(B)), trace=trace,
                                **kwargs)


def kernel(pad_mask, query, key, value, Wq, bq, Wk, bk, Wv, bv):
    in_maps = _prep_inputs(pad_mask, query, key, value, Wq, bq, Wk, bk, Wv, bv)
    res = _run(in_maps)
    out = np.stack([np.asarray(res.results[i]["out"]) for i in range(B)])
    return np.ascontiguousarray(out.astype(np.float32))


# revision 12
# speedup vs baseline: 1.1917x; 1.1805x over previous
"""Trainium2 Bass/Tile kernel: single-head attention (B=8, S=2048, E=1024, DQ=DV=128).

Data-parallel over the batch: one batch element per NeuronCore (8 cores), no
collectives. Host pre-transposes activations to [E, S] bf16 so the contraction
dim lands on SBUF partitions; everything else runs on-chip:

  qT/kT/vT = W.T @ xT          (PE, bf16 in / fp32 PSUM accum, bias added on DVE copy)
  v_aug    = transpose(vT) ++ ones column   (PE transpose; ones column makes the
                                             AV matmul emit softmax row sums for free)
  scoresT  = kT_chunk.T @ qT   ([keys, queries] layout; causal upper blocks skipped)
  attnT    = exp(scoresT/sqrt(DQ) + pad_bias)  (ACT; pad mask is a per-partition bias;
                                               no max-subtraction needed: |scores| < ~3)
  out[q,:] = (attnT.T @ v_aug)[:, :DV] * recip(row_sum)   (PE + DVE recip/scale)

Engine budget per core: PE ~45us, DMA ~36us (13.4MB), ACT ~21us (exp only),
DVE ~19us (copies, masks, normalize).
"""

import numpy as np
import ml_dtypes
from contextlib import ExitStack

B, S, E, DQ, DV = 8, 2048, 1024, 128, 128
EC = E // 128    # contraction chunks
SC = S // 128    # sequence chunks
QB = 512         # matmul moving-dim block
XB = 1024        # exp batching width (2 PSUM banks)
RSQRT_DQ = 1.0 / float(np.sqrt(DQ))
NEG = np.float32(-1e9)
_BF16 = ml_dtypes.bfloat16

_prog = None


def _build_program():
    import concourse.bacc as bacc
    import concourse.mybir as mybir
    import concourse.tile as tile

    f32 = mybir.dt.float32
    bf16 = mybir.dt.bfloat16
    AF = mybir.ActivationFunctionType
    ALU = mybir.AluOpType

    nc = bacc.Bacc("TRN2", target_bir_lowering=False, debug=False)

    d_x = {n: nc.dram_tensor(n, [E, S], bf16, kind="ExternalInput").ap()
           for n in ("qT", "kT", "vT")}
    d_w = {n: nc.dram_tensor(n, [128, EC, 128], bf16, kind="ExternalInput").ap()
           for n in ("wq", "wk", "wv")}
    d_b = {n: nc.dram_tensor(n, [128, 1], f32, kind="ExternalInput").ap()
           for n in ("bq", "bk", "bv")}
    d_padb = nc.dram_tensor("padb", [128, SC], f32, kind="ExternalInput").ap()
    d_tri = nc.dram_tensor("tri", [128, 128], bf16, kind="ExternalInput").ap()
    d_eye = nc.dram_tensor("eye", [128, 128], bf16, kind="ExternalInput").ap()
    d_out = nc.dram_tensor("out", [S, DV], f32, kind="ExternalOutput").ap()

    with tile.TileContext(nc) as tc, ExitStack() as ctx:
        consts = ctx.enter_context(tc.tile_pool(name="consts", bufs=1))
        xin_p = ctx.enter_context(tc.tile_pool(name="xin", bufs=EC // 2 + 1))
        proj_p = ctx.enter_context(tc.tile_pool(name="proj", bufs=1))
        attn_p = ctx.enter_context(tc.tile_pool(name="attn", bufs=1))
        out_p = ctx.enter_context(tc.tile_pool(name="outp", bufs=3))
        # PSUM: proj/vtrans 2 banks + scores 4 + AV 2 = 8
        ps_main = ctx.enter_context(tc.tile_pool(name="ps_main", bufs=2, space="PSUM"))
        ps_sc = ctx.enter_context(tc.tile_pool(name="ps_sc", bufs=2, space="PSUM"))
        ps_av = ctx.enter_context(tc.tile_pool(name="ps_av", bufs=2, space="PSUM"))

        # Big input streams first (HWDGE via sync engine), 1MB per DMA
        # (two E-chunks each) so the stream runs near HBM line rate.
        xin = {}
        for name in ("qT", "kT", "vT"):
            xin[name] = []
            for c2 in range(EC // 2):
                t = xin_p.tile([128, 2, S], bf16, tag="xin")
                src = d_x[name][c2 * 256:(c2 + 1) * 256, :] \
                    .rearrange("(r p) s -> p r s", p=128)
                nc.sync.dma_start(t[:, :, :], src)
                xin[name].append(t)

        # Constants ride the scalar-engine HWDGE ring, off the sync ring.
        w_sb = {}
        for n in ("wq", "wk", "wv"):
            t = consts.tile([128, EC, 128], bf16, tag=n)
            nc.scalar.dma_start(t[:, :, :], d_w[n])
            w_sb[n] = t
        b_sb = {}
        for n in ("bq", "bk", "bv"):
            t = consts.tile([128, 1], f32, tag=n)
            nc.scalar.dma_start(t[:, :], d_b[n])
            b_sb[n] = t
        padb = consts.tile([128, SC], f32, tag="padb")
        nc.scalar.dma_start(padb[:, :], d_padb)
        tri = consts.tile([128, 128], bf16, tag="tri")
        nc.scalar.dma_start(tri[:, :], d_tri)
        eye = consts.tile([128, 128], bf16, tag="eye")
        nc.scalar.dma_start(eye[:, :], d_eye)

        # One-time exp LUT load, hidden under the first input DMAs.
        warm = consts.tile([128, 1], f32, tag="warm")
        nc.vector.memset(warm[:, :], 0.0)
        wo = consts.tile([128, 1], f32, tag="warmo")
        nc.scalar.activation(wo[:, :], warm[:, :], AF.Exp)

        # ---- Projections: xT [E, S] -> qT/kT/vT [128(dq/dv), S] bf16 ----
        qT = proj_p.tile([128, S], bf16, tag="qT")
        kT = proj_p.tile([128, S], bf16, tag="kT")
        vT = proj_p.tile([128, S], bf16, tag="vT")
        for name, bias, dst in (("qT", b_sb["bq"], qT),
                                ("kT", b_sb["bk"], kT),
                                ("vT", b_sb["bv"], vT)):
            w = w_sb["w" + name[0]]
            for n0 in range(0, S, QB):
                ps = ps_main.tile([128, QB], f32, tag="ps")
                for c in range(EC):
                    nc.tensor.matmul(ps[:, :], w[:, c, :],
                                     xin[name][c // 2][:, c % 2, n0:n0 + QB],
                                     start=(c == 0), stop=(c == EC - 1))
                # copy + per-partition bias add + bf16 cast on DVE
                nc.vector.tensor_scalar(dst[:, n0:n0 + QB], ps[:, :],
                                        bias[:, :], None, ALU.add)

        # ---- v_aug[j] = v natural [keys, DV] ++ ones column ----
        vaug = []
        for j in range(SC):
            ps = ps_main.tile([128, 128], bf16, tag="ps")
            nc.tensor.transpose(ps[:, :], vT[:, j * 128:(j + 1) * 128], eye[:, :])
            va = attn_p.tile([128, DV + 1], bf16, tag=f"vaug{j}")
            nc.vector.tensor_copy(va[:, 0:DV], ps[:, :])
            nc.vector.memset(va[:, DV:DV + 1], 1.0)
            vaug.append(va)

        # ---- scoresT -> exp -> attnT bf16, per key chunk j (causal: q >= j*128) ----
        attnT = []
        for j in range(SC):
            at = attn_p.tile([128, S - j * 128], bf16, tag=f"attnT{j}")
            attnT.append(at)
            p0 = j * 128
            while p0 < S:
                n = min(XB, S - p0)
                ps = ps_sc.tile([128, n], f32, tag="ps_sc")
                for q0 in range(p0, p0 + n, QB):
                    m = min(QB, p0 + n - q0)
                    nc.tensor.matmul(ps[:, q0 - p0:q0 - p0 + m],
                                     kT[:, j * 128:(j + 1) * 128],
                                     qT[:, q0:q0 + m], start=True, stop=True)
                nc.scalar.activation(at[:, p0 - j * 128:p0 - j * 128 + n], ps[:, :],
                                     AF.Exp, bias=padb[:, j:j + 1], scale=RSQRT_DQ)
                p0 += n
            # in-block causal mask on the diagonal block (keep k <= q)
            nc.vector.tensor_mul(at[:, 0:128], at[:, 0:128], tri[:, :])

        # ---- AV per q tile + fused normalization ----
        for i in range(SC):
            ps = ps_av.tile([128, DV + 1], f32, tag="pso")
            for j in range(i + 1):
                nc.tensor.matmul(ps[:, :],
                                 attnT[j][:, (i - j) * 128:(i - j) * 128 + 128],
                                 vaug[j][:, :], start=(j == 0), stop=(j == i))
            rec = out_p.tile([128, 1], f32, tag="rec")
            nc.vector.reciprocal(rec[:, :], ps[:, DV:DV + 1])
            ot = out_p.tile([128, DV], f32, tag="ot")
            nc.vector.tensor_scalar(ot[:, :], ps[:, 0:DV], rec[:, :], None,
                                    ALU.mult)
            nc.scalar.dma_start(d_out[i * 128:(i + 1) * 128, :], ot[:, :])

    nc.compile()
    return nc


def _prep_inputs(pad_mask, query, key, value, Wq, bq, Wk, bk, Wv, bv):
    def wprep(w):
        return np.ascontiguousarray(
            np.asarray(w, np.float32).astype(_BF16).reshape(EC, 128, 128)
            .transpose(1, 0, 2))

    def bprep(v):
        return np.ascontiguousarray(np.asarray(v, np.float32).reshape(128, 1))

    shared = {
        "wq": wprep(Wq), "wk": wprep(Wk), "wv": wprep(Wv),
        "bq": bprep(bq), "bk": bprep(bk), "bv": bprep(bv),
        "tri": np.triu(np.ones((128, 128), np.float32)).astype(_BF16),
        "eye": np.eye(128, dtype=np.float32).astype(_BF16),
    }
    pad_mask = np.asarray(pad_mask)
    query = np.asarray(query, np.float32)
    key = np.asarray(key, np.float32)
    value = np.asarray(value, np.float32)
    in_maps = []
    for b in range(B):
        padb = np.ascontiguousarray(
            np.where(pad_mask[b], NEG, np.float32(0.0)).reshape(SC, 128).T)
        in_maps.append({
            **shared,
            "qT": query[b].T.astype(_BF16, order="C"),
            "kT": key[b].T.astype(_BF16, order="C"),
            "vT": value[b].T.astype(_BF16, order="C"),
            "padb": padb.astype(np.float32),
        })
    return in_maps


def _run(in_maps, trace=False, **kwargs):
    global _prog
    from concourse.bass_utils import run_bass_kernel_spmd
    if _prog is None:
        _prog = _build_program()
    return run_bass_kernel_spmd(_prog, in_maps, list(range(B)), trace=trace,
                                **kwargs)


def kernel(pad_mask, query, key, value, Wq, bq, Wk, bk, Wv, bv):
    in_maps = _prep_inputs(pad_mask, query, key, value, Wq, bq, Wk, bk, Wv, bv)
    res = _run(in_maps)
    out = np.stack([np.asarray(res.results[i]["out"]) for i in range(B)])
    return np.ascontiguousarray(out.astype(np.float32))


# revision 15
# speedup vs baseline: 1.3019x; 1.0925x over previous
"""Trainium2 Bass/Tile kernel: single-head attention (B=8, S=2048, E=1024, DQ=DV=128).

Data-parallel over the batch: one batch element per NeuronCore (8 cores), no
collectives. Host pre-transposes activations to [E, S] bf16 so the contraction
dim lands on SBUF partitions; everything else runs on-chip:

  qT/kT/vT = W.T @ xT          (PE, bf16 in / fp32 PSUM accum, bias added on DVE copy)
  v_aug    = transpose(vT) ++ ones column   (PE transpose; ones column makes the
                                             AV matmul emit softmax row sums for free)
  scoresT  = kT_chunk.T @ qT   ([keys, queries] layout; causal upper blocks skipped)
  attnT    = exp(scoresT/sqrt(DQ) + pad_bias)  (ACT; pad mask is a per-partition bias;
                                               no max-subtraction needed: |scores| < ~3)
  out[q,:] = (attnT.T @ v_aug)[:, :DV] * recip(row_sum)   (PE + DVE recip/scale)

Trace order == engine FIFO order, so it is arranged to match the ideal
timeline: q/k streams (parallel HWDGE rings) -> q proj -> k proj interleaved
with scores+exp -> v proj/transpose -> AV.  This keeps PE dense (HAM warm) and
overlaps the attention math with the tail of the input DMA stream.
"""

import numpy as np
import ml_dtypes
from contextlib import ExitStack

B, S, E, DQ, DV = 8, 2048, 1024, 128, 128
EC = E // 128    # contraction chunks
SC = S // 128    # sequence chunks
QB = 512         # matmul moving-dim block
XB = 1024        # exp batching width (2 PSUM banks)
RSQRT_DQ = 1.0 / float(np.sqrt(DQ))
NEG = np.float32(-1e9)
_BF16 = ml_dtypes.bfloat16

_prog = None


def _build_program():
    import concourse.bacc as bacc
    import concourse.mybir as mybir
    import concourse.tile as tile

    f32 = mybir.dt.float32
    bf16 = mybir.dt.bfloat16
    AF = mybir.ActivationFunctionType
    ALU = mybir.AluOpType

    nc = bacc.Bacc("TRN2", target_bir_lowering=False, debug=False)

    d_x = {n: nc.dram_tensor(n, [E, S], bf16, kind="ExternalInput").ap()
           for n in ("qT", "kT", "vT")}
    d_w = {n: nc.dram_tensor(n, [128, EC, 128], bf16, kind="ExternalInput").ap()
           for n in ("wq", "wk", "wv")}
    d_b = {n: nc.dram_tensor(n, [128, 1], f32, kind="ExternalInput").ap()
           for n in ("bq", "bk", "bv")}
    d_padb = nc.dram_tensor("padb", [128, SC], f32, kind="ExternalInput").ap()
    d_tri = nc.dram_tensor("tri", [128, 128], bf16, kind="ExternalInput").ap()
    d_eye = nc.dram_tensor("eye", [128, 128], bf16, kind="ExternalInput").ap()
    d_out = nc.dram_tensor("out", [S, DV], f32, kind="ExternalOutput").ap()

    with tile.TileContext(nc) as tc, ExitStack() as ctx:
        consts = ctx.enter_context(tc.tile_pool(name="consts", bufs=1))
        xin_p = ctx.enter_context(tc.tile_pool(name="xin", bufs=EC // 2))
        proj_p = ctx.enter_context(tc.tile_pool(name="proj", bufs=1))
        attn_p = ctx.enter_context(tc.tile_pool(name="attn", bufs=1))
        out_p = ctx.enter_context(tc.tile_pool(name="outp", bufs=3))
        # PSUM budget: proj/vtrans 2 banks + scores 4 + AV 2 = 8
        ps_main = ctx.enter_context(tc.tile_pool(name="ps_main", bufs=2, space="PSUM"))
        ps_sc = ctx.enter_context(tc.tile_pool(name="ps_sc", bufs=2, space="PSUM"))
        ps_av = ctx.enter_context(tc.tile_pool(name="ps_av", bufs=2, space="PSUM"))

        def xin_dma(eng, name, c2, tag):
            t = xin_p.tile([128, 2, S], bf16, tag=tag)
            src = d_x[name][c2 * 256:(c2 + 1) * 256, :] \
                .rearrange("(r p) s -> p r s", p=128)
            eng.dma_start(t[:, :, :], src)
            return t

        # q stream on the sync HWDGE ring; consts then k stream on the
        # scalar HWDGE ring (parallel); v stream split across both.
        xq = [xin_dma(nc.sync, "qT", c2, "xq") for c2 in range(EC // 2)]

        w_sb = {}
        for n in ("wq", "wk", "wv"):
            t = consts.tile([128, EC, 128], bf16, tag=n)
            nc.scalar.dma_start(t[:, :, :], d_w[n])
            w_sb[n] = t
        b_sb = {}
        for n in ("bq", "bk", "bv"):
            t = consts.tile([128, 1], f32, tag=n)
            nc.scalar.dma_start(t[:, :], d_b[n])
            b_sb[n] = t
        padb = consts.tile([128, SC], f32, tag="padb")
        nc.scalar.dma_start(padb[:, :], d_padb)
        tri = consts.tile([128, 128], bf16, tag="tri")
        nc.scalar.dma_start(tri[:, :], d_tri)
        eye = consts.tile([128, 128], bf16, tag="eye")
        nc.scalar.dma_start(eye[:, :], d_eye)

        xk = [xin_dma(nc.scalar, "kT", c2, "xk") for c2 in range(EC // 2)]
        xv = [xin_dma((nc.sync, nc.scalar)[c2 % 2], "vT", c2, "xv")
              for c2 in range(EC // 2)]
        xin = {"qT": xq, "kT": xk, "vT": xv}

        # One-time exp LUT load, hidden under the first input DMAs.
        warm = consts.tile([128, 1], f32, tag="warm")
        nc.vector.memset(warm[:, :], 0.0)
        wo = consts.tile([128, 1], f32, tag="warmo")
        nc.scalar.activation(wo[:, :], warm[:, :], AF.Exp)

        qT = proj_p.tile([128, S], bf16, tag="qT")
        kT = proj_p.tile([128, S], bf16, tag="kT")
        vT = proj_p.tile([128, S], bf16, tag="vT")

        def proj_piece(name, bias, dst, n0):
            w = w_sb["w" + name[0]]
            ps = ps_main.tile([128, QB], f32, tag="ps")
            for c in range(EC):
                nc.tensor.matmul(ps[:, :], w[:, c, :],
                                 xin[name][c // 2][:, c % 2, n0:n0 + QB],
                                 start=(c == 0), stop=(c == EC - 1))
            # copy + per-partition bias add + bf16 cast on DVE
            nc.vector.tensor_scalar(dst[:, n0:n0 + QB], ps[:, :],
                                    bias[:, :], None, ALU.add)

        def scores_chunk(j, at):
            # scoresT[j] -> exp -> attnT[j] bf16 (causal: q >= j*128)
            p0 = j * 128
            while p0 < S:
                n = min(XB, S - p0)
                ps = ps_sc.tile([128, n], f32, tag="ps_sc")
                for q0 in range(p0, p0 + n, QB):
                    m = min(QB, p0 + n - q0)
                    nc.tensor.matmul(ps[:, q0 - p0:q0 - p0 + m],
                                     kT[:, j * 128:(j + 1) * 128],
                                     qT[:, q0:q0 + m], start=True, stop=True)
                nc.scalar.activation(at[:, p0 - j * 128:p0 - j * 128 + n],
                                     ps[:, :], AF.Exp,
                                     bias=padb[:, j:j + 1], scale=RSQRT_DQ)
                p0 += n
            # in-block causal mask on the diagonal block (keep k <= q)
            nc.vector.tensor_mul(at[:, 0:128], at[:, 0:128], tri[:, :])

        # ---- q projection ----
        for n0 in range(0, S, QB):
            proj_piece("qT", b_sb["bq"], qT, n0)

        # ---- k projection interleaved with scores for the ready key chunks ----
        attnT = [attn_p.tile([128, S - j * 128], bf16, tag=f"attnT{j}",
                             name=f"attnT{j}")
                 for j in range(SC)]
        for n0 in range(0, S, QB):
            proj_piece("kT", b_sb["bk"], kT, n0)
            for j in range(n0 // 128, n0 // 128 + 4):
                scores_chunk(j, attnT[j])

        # ---- v projection, then v_aug[j] = v natural [keys, DV] ++ ones ----
        for n0 in range(0, S, QB):
            proj_piece("vT", b_sb["bv"], vT, n0)
        vaug = []
        for j in range(SC):
            ps = ps_main.tile([128, 128], bf16, tag="ps")
            nc.tensor.transpose(ps[:, :], vT[:, j * 128:(j + 1) * 128], eye[:, :])
            va = attn_p.tile([128, DV + 1], bf16, tag=f"vaug{j}")
            nc.vector.tensor_copy(va[:, 0:DV], ps[:, :])
            nc.vector.memset(va[:, DV:DV + 1], 1.0)
            vaug.append(va)

        # ---- AV per q tile + fused normalization ----
        for i in range(SC):
            ps = ps_av.tile([128, DV + 1], f32, tag="pso")
            for j in range(i + 1):
                nc.tensor.matmul(ps[:, :],
                                 attnT[j][:, (i - j) * 128:(i - j) * 128 + 128],
                                 vaug[j][:, :], start=(j == 0), stop=(j == i))
            rec = out_p.tile([128, 1], f32, tag="rec")
            nc.vector.reciprocal(rec[:, :], ps[:, DV:DV + 1])
            ot = out_p.tile([128, DV], f32, tag="ot")
            nc.vector.tensor_scalar(ot[:, :], ps[:, 0:DV], rec[:, :], None,
                                    ALU.mult)
            nc.sync.dma_start(d_out[i * 128:(i + 1) * 128, :], ot[:, :])

    nc.compile()
    return nc


def _prep_inputs(pad_mask, query, key, value, Wq, bq, Wk, bk, Wv, bv):
    def wprep(w):
        return np.ascontiguousarray(
            np.asarray(w, np.float32).astype(_BF16).reshape(EC, 128, 128)
            .transpose(1, 0, 2))

    def bprep(v):
        return np.ascontiguousarray(np.asarray(v, np.float32).reshape(128, 1))

    shared = {
        "wq": wprep(Wq), "wk": wprep(Wk), "wv": wprep(Wv),
        "bq": bprep(bq), "bk": bprep(bk), "bv": bprep(bv),
        "tri": np.triu(np.ones((128, 128), np.float32)).astype(_BF16),
        "eye": np.eye(128, dtype=np.float32).astype(_BF16),
    }
    pad_mask = np.asarray(pad_mask)
    query = np.asarray(query, np.float32)
    key = np.asarray(key, np.float32)
    value = np.asarray(value, np.float32)
    in_maps = []
    for b in range(B):
        padb = np.ascontiguousarray(
            np.where(pad_mask[b], NEG, np.float32(0.0)).reshape(SC, 128).T)
        in_maps.append({
            **shared,
            "qT": query[b].T.astype(_BF16, order="C"),
            "kT": key[b].T.astype(_BF16, order="C"),
            "vT": value[b].T.astype(_BF16, order="C"),
            "padb": padb.astype(np.float32),
        })
    return in_maps


def _run(in_maps, trace=False, **kwargs):
    global _prog
    from concourse.bass_utils import run_bass_kernel_spmd
    if _prog is None:
        _prog = _build_program()
    return run_bass_kernel_spmd(_prog, in_maps, list(range(B)), trace=trace,
                                **kwargs)


def kernel(pad_mask, query, key, value, Wq, bq, Wk, bk, Wv, bv):
    in_maps = _prep_inputs(pad_mask, query, key, value, Wq, bq, Wk, bk, Wv, bv)
    res = _run(in_maps)
    out = np.stack([np.asarray(res.results[i]["out"]) for i in range(B)])
    return np.ascontiguousarray(out.astype(np.float32))
